# revision 12
# baseline (speedup 1.0000x reference)
"""ClassAwareTripletLoss Trainium2 kernel (8 NeuronCores, data-parallel over batch).

Math (pos_prot rows are unit-norm, x = inputs/||inputs||):
  prot_dist[b,c,k]^2 = 2 - 2 * x.p_k          (x2 = p2 = 1)
  neg_cls = argmax_{k != c} x.p_k
  d_an = sqrt(2 - 2 * max_{k != c} (x_raw.p_k) / nrm)
  d_ap = sqrt(2 - 2 * (x_raw.p_c) / nrm)
  loss = mean_b( sum_c relu(d_ap - d_an + 0.2) * w / sum_c w )
(The PAIR_EPS=1e-6 inside torch pairwise distances perturbs the result by
 ~1e-5 relative and is dropped.)

Per core (8 samples): bf16 matmul x^T @ protT -> PSUM [128,1024] per
(sample, c-tile) unit; self-class excluded by accumulating -BIG*I into the
diagonal block with one extra matmul.  The PSUM drain (row-max over 1024
prototypes) is split between VectorE (true reduce_max) and ScalarE
(exp-sum LSE: max ~= 1 + ln(sum_k exp(beta*(s/nrm - 1) + SHIFT))/beta - SHIFT/beta,
beta=100; SHIFT keeps the accumulator inside ACT-Ln's usable domain) so both
engines' PSUM ports run in parallel.  Sample pairs are stacked in partition
halves so xbar DMA-transposes are legal and matmuls can row-pack the PE array.
"""

import numpy as np
from contextlib import ExitStack

import concourse.bass as bass
import concourse.bacc as bacc
import concourse.tile as tile
from concourse import mybir
from concourse.bass_utils import run_bass_kernel_spmd

f32 = mybir.dt.float32
bf16 = mybir.dt.bfloat16
AL = mybir.AluOpType
AF = mybir.ActivationFunctionType

BS, C, D = 64, 1024, 64
NCORES = 8
BSL = BS // NCORES          # 8 samples per core
T = C // 128                # 8 c-tiles of 128
NUNITS = T * BSL            # 64 (t, b) units; column index = t*8 + b
BETA = 100.0
BIG = 30000.0
MARGIN = 0.2
LSE_SHIFT = 46.0            # keeps the LSE accumulator within ACT-Ln's domain
N_ACT = 40                  # units drained on ScalarE via LSE (cols 0..N_ACT-1)
                            # remaining NUNITS-N_ACT drained on VectorE reduce_max


def _col(t, b):
    return t * BSL + b


def build(n_act=N_ACT, debug_taps=False, reps=1):
    nc = bacc.Bacc("TRN2", target_bir_lowering=False, debug=False)
    x_d = nc.dram_tensor("inputs", [BSL, C, D], f32, kind="ExternalInput")
    lab_d = nc.dram_tensor("label", [BSL, C], f32, kind="ExternalInput")
    prot_d = nc.dram_tensor("pos_prot", [C, D], f32, kind="ExternalInput")
    out_d = nc.dram_tensor("out", [NUNITS, 2], f32, kind="ExternalOutput")
    if debug_taps:
        tap_d = {name: nc.dram_tensor("tap_" + name, [128, NUNITS], f32,
                                      kind="ExternalOutput")
                 for name in ("inv_nrm", "dd", "md", "mx", "acc", "d_ap", "d_an")}

    with tile.TileContext(nc) as tc, ExitStack() as ctx:
        CP = ctx.enter_context(tc.tile_pool(name="const", bufs=1))
        P = ctx.enter_context(tc.tile_pool(name="persist", bufs=1))
        scrp = ctx.enter_context(tc.tile_pool(name="scr", bufs=3))
        prodp = ctx.enter_context(tc.tile_pool(name="prod", bufs=2))
        psp = ctx.enter_context(tc.tile_pool(name="ps", bufs=3, space="PSUM"))
        pse = ctx.enter_context(tc.tile_pool(name="pse", bufs=1, space="PSUM"))

        # ---- constants (one-time) --------------------------------------
        onesf = CP.tile([128, 1], f32)
        nc.vector.memset(onesf, 1.0)
        nbeta = CP.tile([128, 1], f32)
        nc.vector.memset(nbeta, -(BETA - LSE_SHIFT))
        one128 = CP.tile([128, 128], f32)
        nc.vector.memset(one128, 1.0)
        eyef = CP.tile([128, 128], f32)
        # keep 1.0 where (j - p) == 0 else 0 -> identity
        nc.gpsimd.affine_select(eyef, one128, pattern=[[1, 128]],
                                compare_op=AL.is_equal, fill=0.0,
                                base=0, channel_multiplier=-1)
        eyeb = CP.tile([128, 128], bf16)
        nc.vector.tensor_copy(eyeb, eyef)
        negib = CP.tile([128, 128], bf16)
        nc.vector.tensor_scalar_mul(negib, eyef, -BIG)

        # ---- prototype load / transpose (one-time) ---------------------
        # pr[p, (t d)] = prot[t*128+p, d]
        pr = CP.tile([128, T, D], f32)
        nc.sync.dma_start(out=pr, in_=prot_d.ap().rearrange("(t p) d -> p t d", p=128))
        prb = CP.tile([128, T, D], bf16)
        nc.vector.tensor_copy(prb, pr)
        # duplicate prot along a pair axis so its transpose fills both
        # partition halves (xbar transpose needs free %128 == 0)
        prb2 = CP.tile([128, T, 2, D], bf16)
        nc.vector.tensor_copy(prb2[:, :, 0, :], prb)
        nc.vector.tensor_copy(prb2[:, :, 1, :], prb)
        # protT2[d + 64*half, k] = prot[k, d] for half in {0, 1}
        protT2 = CP.tile([128, C], bf16)
        for t in range(T):
            nc.sync.dma_start_transpose(
                out=protT2[:, t * 128:(t + 1) * 128],
                in_=prb2[:, t, :, :].rearrange("p a d -> p (a d)"))

        def emit_rep():
            # ---- per-sample input loads and derived tiles --------------
            # xf[p, (t b d)] = inputs[b, t*128+p, d]
            xf = P.tile([128, T, BSL, D], f32, tag="xf")
            xbf = P.tile([128, T, BSL, D], bf16, tag="xbf")
            sqb = P.tile([128, T, BSL, D], bf16, tag="sqb")
            # xT2[d + 64*(b%2), b//2, c] = x[b, c, d] (bf16)
            xT2 = P.tile([128, BSL // 2, C], bf16, tag="xT2")
            w = P.tile([128, NUNITS], f32, tag="w")
            nrm2 = P.tile([128, NUNITS], f32, tag="nrm2")
            inv_nrm = P.tile([128, NUNITS], f32, tag="inv_nrm")
            scl = P.tile([128, NUNITS], f32, tag="scl")
            acc = P.tile([128, NUNITS], f32, tag="acc")
            mx = P.tile([128, NUNITS], f32, tag="mx")
            md = P.tile([128, NUNITS], f32, tag="md")
            dd = P.tile([128, NUNITS], f32, tag="dd")

            for b in range(BSL):
                nc.sync.dma_start(
                    out=xf[:, :, b, :],
                    in_=x_d.ap()[b].rearrange("(t p) d -> p t d", p=128))
                nc.sync.dma_start(
                    out=w[:, b::BSL],
                    in_=lab_d.ap()[b].rearrange("(t p) -> p t", p=128))
                # bf16 cast (DVE 2x) + squares (DVE bf16 2x) + row-norm reduce
                nc.vector.tensor_copy(xbf[:, :, b, :], xf[:, :, b, :])
                nc.vector.tensor_mul(sqb[:, :, b, :], xbf[:, :, b, :], xbf[:, :, b, :])
                nc.vector.reduce_sum(
                    out=nrm2[:, b::BSL],
                    in_=sqb[:, :, b, :], axis=mybir.AxisListType.X)
                # inv_nrm = exp(-0.5 * ln(nrm2)); scl = BETA * inv_nrm
                nc.scalar.activation(inv_nrm[:, b::BSL], nrm2[:, b::BSL], AF.Ln)
                nc.scalar.activation(inv_nrm[:, b::BSL], inv_nrm[:, b::BSL],
                                     AF.Exp, scale=-0.5)
                nc.vector.tensor_scalar_mul(scl[:, b::BSL], inv_nrm[:, b::BSL], BETA)
                # transpose a pair of samples at a time once both are cast
                if b % 2 == 1:
                    j = b // 2
                    for t in range(T):
                        nc.sync.dma_start_transpose(
                            out=xT2[:, j, t * 128:(t + 1) * 128],
                            in_=xbf[:, t, b - 1:b + 1, :].rearrange("p a d -> p (a d)"))

            # ---- main matmuls + drains --------------------------------
            # sample pairs share the PE array: even sample in rows 0-63,
            # odd sample in rows 64-127 (tile_position via base_partition)
            for j in range(BSL // 2):
                for t in range(T):
                    diag_half = 0 if t < T // 2 else 1
                    lo = t * 128 - diag_half * 512
                    pss = []
                    for half in range(2):
                        ps = psp.tile([128, 2, 512], f32, tag="psu")
                        pss.append(ps)
                        lhsT = xT2[64 * half:64 * (half + 1), j, t * 128:(t + 1) * 128]
                        rhs = protT2[64 * half:64 * (half + 1), :]
                        h0 = diag_half
                        nc.tensor.matmul(ps[:, h0, :], lhsT,
                                         rhs[:, h0 * 512:(h0 + 1) * 512],
                                         start=True, stop=False, skip_group_check=True)
                        # exclude self-class: accumulate -BIG onto the diag block
                        nc.tensor.matmul(ps[:, h0, lo:lo + 128], eyeb, negib,
                                         start=False, stop=True, skip_group_check=True)
                        h1 = 1 - diag_half
                        nc.tensor.matmul(ps[:, h1, :], lhsT,
                                         rhs[:, h1 * 512:(h1 + 1) * 512],
                                         start=True, stop=True, skip_group_check=True)
                    for half in range(2):
                        col = _col(t, 2 * j + half)
                        flat = pss[half].rearrange("p a n -> p (a n)")
                        if col < n_act:
                            scr = scrp.tile([128, 1024], bf16, tag="scr")
                            nc.scalar.activation(scr, flat, AF.Exp,
                                                 bias=nbeta, scale=scl[:, col:col + 1],
                                                 accum_out=acc[:, col:col + 1])
                        else:
                            nc.vector.reduce_max(out=mx[:, col:col + 1], in_=flat,
                                                 axis=mybir.AxisListType.X)

            # ---- dd[b,c] = inputs[b,c,:].prot[c,:] (bf16) --------------
            for b in range(BSL):
                prod = prodp.tile([128, T, D], bf16, tag="prod")
                nc.vector.tensor_mul(prod, xbf[:, :, b, :], prb)
                nc.vector.reduce_sum(
                    out=dd[:, b::BSL],
                    in_=prod, axis=mybir.AxisListType.X)

            # ---- epilogue ([128, 64] tiles) ----------------------------
            # maxdot (normalized): ACT cols via LSE, DVE cols via mx * inv_nrm
            nc.scalar.activation(md[:, :n_act], acc[:, :n_act], AF.Ln)
            nc.vector.tensor_scalar(md[:, :n_act], md[:, :n_act],
                                    1.0 / BETA, 1.0 - LSE_SHIFT / BETA,
                                    AL.mult, AL.add)
            if n_act < NUNITS:
                nc.vector.tensor_mul(md[:, n_act:], mx[:, n_act:], inv_nrm[:, n_act:])

            d_an = P.tile([128, NUNITS], f32, tag="d_an")
            d_ap = P.tile([128, NUNITS], f32, tag="d_ap")
            # d_an = sqrt(max(2 - 2*md, eps)) = exp(0.5*ln(...))
            nc.vector.tensor_scalar(d_an, md, -2.0, 2.0, AL.mult, AL.add)
            nc.vector.tensor_scalar_max(d_an, d_an, 1e-20)
            nc.scalar.activation(d_an, d_an, AF.Ln)
            nc.scalar.activation(d_an, d_an, AF.Exp, scale=0.5)
            # d_ap = sqrt(max(2 - 2*dd*inv_nrm, eps))
            nc.vector.tensor_mul(d_ap, dd, inv_nrm)
            nc.vector.tensor_scalar(d_ap, d_ap, -2.0, 2.0, AL.mult, AL.add)
            nc.vector.tensor_scalar_max(d_ap, d_ap, 1e-20)
            nc.scalar.activation(d_ap, d_ap, AF.Ln)
            nc.scalar.activation(d_ap, d_ap, AF.Exp, scale=0.5)

            # triw = relu(d_ap + MARGIN - d_an) * w
            pre = P.tile([128, NUNITS], f32, tag="pre")
            nc.vector.scalar_tensor_tensor(pre, d_ap, MARGIN, d_an, AL.add, AL.subtract)
            triw = P.tile([128, NUNITS], f32, tag="triw")
            nc.vector.scalar_tensor_tensor(triw, pre, 0.0, w, AL.max, AL.mult)

            # per-(t,b) partition sums via ones-matmul
            pnum = pse.tile([NUNITS, 1], f32, tag="pnum")
            pden = pse.tile([NUNITS, 1], f32, tag="pden")
            nc.tensor.matmul(pnum, triw, onesf, start=True, stop=True)
            nc.tensor.matmul(pden, w, onesf, start=True, stop=True)
            outsb = P.tile([NUNITS, 2], f32, tag="outsb")
            nc.vector.tensor_copy(outsb[:, 0:1], pnum)
            nc.vector.tensor_copy(outsb[:, 1:2], pden)
            nc.sync.dma_start(out=out_d.ap(), in_=outsb)
            if debug_taps:
                taps = dict(inv_nrm=inv_nrm, dd=dd, md=md, mx=mx, acc=acc,
                            d_ap=d_ap, d_an=d_an)
                for name, t_ in taps.items():
                    nc.sync.dma_start(out=tap_d[name].ap(), in_=t_)

        for _ in range(reps):
            emit_rep()

    nc.compile()
    return nc


_NC = None


def _get_nc():
    global _NC
    if _NC is None:
        _NC = build()
    return _NC


def make_in_maps(inputs, label, pos_prot):
    in_maps = []
    for i in range(NCORES):
        in_maps.append({
            "inputs": np.ascontiguousarray(inputs[i * BSL:(i + 1) * BSL], np.float32),
            "label": np.ascontiguousarray(label[i * BSL:(i + 1) * BSL, :, 0], np.float32),
            "pos_prot": np.ascontiguousarray(pos_prot, np.float32),
        })
    return in_maps


def run_cores(inputs, label, pos_prot):
    nc = _get_nc()
    return run_bass_kernel_spmd(nc, make_in_maps(inputs, label, pos_prot),
                                core_ids=list(range(NCORES)))


def finish(res):
    per_sample = []
    for i in range(NCORES):
        o = res.results[i]["out"].reshape(T, BSL, 2)
        num = o[:, :, 0].sum(axis=0, dtype=np.float64)
        den = o[:, :, 1].sum(axis=0, dtype=np.float64)
        per_sample.append(num / den)
    return np.float32(np.mean(np.concatenate(per_sample)))


def kernel(inputs, label, pos_prot, only_update=0, **_unused):
    res = run_cores(np.asarray(inputs), np.asarray(label), np.asarray(pos_prot))
    return finish(res)
